# revision 1
# baseline (speedup 1.0000x reference)
"""Llama block (single-token decode) on 8 TRN2 NeuronCores, tensor-parallel.

Sharding (per core c of 8):
  - heads 4c..4c+3: w_q/w_k/w_v column shards [4096, 512], KV cache [4096, 4, 128]
  - w_o row shard [512, 4096] -> partial attn output, AllReduce'd on device
  - w_ff1 column shard [4096, 1376], w_ff2 row shard [1376, 4096]
  - per-core FFN partials summed on host (row-sharded output unshard)

On-chip layout convention: a length-4096 vector is held as [128, 32] "cols"
(element (p, t) = vec[t*128 + p]) so vector tiles feed matmul lhsT directly.
PSUM discipline: every accumulation group writes one fixed region repeatedly
(same-region WAW keeps the scheduler honest); cross-layout moves go through
small SBUF->SBUF DMAs rather than PE transposes.
"""

import math
import sys

sys.path.insert(0, "/opt/trn_rl_repo")

import numpy as np

import concourse.bass as bass
import concourse.tile as tile
from concourse import bacc, mybir
from concourse.bass_utils import run_bass_kernel_spmd
from concourse import bass_isa

F32 = mybir.dt.float32
AF = mybir.ActivationFunctionType
ALU = mybir.AluOpType
AX = mybir.AxisListType

H = 4096
NH = 32
HD = 128
INTERM = 11008
EPS = 1e-6
CORES = 8
HPC = NH // CORES  # 4 heads per core
QC = HPC * HD  # 512 qkv cols per core
FFC = INTERM // CORES  # 1376 ff cols per core
KT = H // 128  # 32 contraction tiles
SCALE = 1.0 / math.sqrt(HD)

_BUILD_CACHE = {}


def _sb_ap(t, ap, extra_offset=0):
    """Raw AP over an SBUF tile view t (an AP), with explicit dims."""
    return bass.AP(tensor=t.tensor, offset=t.offset + extra_offset, ap=ap)


def _build(pos: int):
    if pos in _BUILD_CACHE:
        return _BUILD_CACHE[pos]

    n_s = pos + 1
    n_tiles = (n_s + 127) // 128  # s-tiles to attend over
    rem = n_s - (n_tiles - 1) * 128  # rows in last s-tile (1..128)
    pos_tile = pos // 128
    pos_row = pos % 128

    nc = bacc.Bacc("TRN2", target_bir_lowering=False, debug=False, num_devices=CORES)

    x_in = nc.dram_tensor("x_cols", [128, KT], F32, kind="ExternalInput")
    an_in = nc.dram_tensor("an_cols", [128, KT], F32, kind="ExternalInput")
    fn_in = nc.dram_tensor("fn_cols", [128, KT], F32, kind="ExternalInput")
    cos_in = nc.dram_tensor("cos4", [QC], F32, kind="ExternalInput")
    sin_in = nc.dram_tensor("sin4", [QC], F32, kind="ExternalInput")
    wq_in = nc.dram_tensor("wq", [H, QC], F32, kind="ExternalInput")
    wk_in = nc.dram_tensor("wk", [H, QC], F32, kind="ExternalInput")
    wv_in = nc.dram_tensor("wv", [H, QC], F32, kind="ExternalInput")
    wo_in = nc.dram_tensor("wo", [QC, H], F32, kind="ExternalInput")
    kc_in = nc.dram_tensor("kc", [H, HPC, HD], F32, kind="ExternalInput")
    vc_in = nc.dram_tensor("vc", [H, HPC, HD], F32, kind="ExternalInput")
    ff1_in = nc.dram_tensor("ff1", [H, FFC], F32, kind="ExternalInput")
    ff2_in = nc.dram_tensor("ff2", [FFC, H], F32, kind="ExternalInput")

    xnew_out = nc.dram_tensor("xnew_out", [128, KT], F32, kind="ExternalOutput")
    ff_out = nc.dram_tensor("ff_out", [H], F32, kind="ExternalOutput")

    # DRAM-side chunked views
    wq_v = wq_in.ap().rearrange("(g j p) n -> g p j n", p=128, j=8)  # 4 chunks
    wk_v = wk_in.ap().rearrange("(g j p) n -> g p j n", p=128, j=8)
    wv_v = wv_in.ap().rearrange("(g j p) n -> g p j n", p=128, j=8)
    kc_v = kc_in.ap().rearrange("(c s p) h d -> c p s (h d)", p=128, s=8)
    vc_v = vc_in.ap().rearrange("(c s p) h d -> c p s (h d)", p=128, s=8)
    wo_v = wo_in.ap().rearrange("(c p) n -> c p n", p=128)  # 4 chunks [128, 4096]
    ff1_v = ff1_in.ap().rearrange("(c j p) n -> c p j n", p=128, j=2)  # 16 chunks
    ff2_v = ff2_in.ap()

    NFF1T = (FFC + 511) // 512  # 3 psum rows for ff1
    FFKT = (FFC + 127) // 128  # 11 contraction tiles for ff2

    with tile.TileContext(nc) as tc:
        with (
            tc.tile_pool(name="stream", bufs=8) as stream,
            tc.tile_pool(name="small", bufs=1) as small,
            tc.tile_pool(name="work", bufs=1) as work,
            tc.tile_pool(name="ps_row", bufs=4, space="PSUM") as ps_row,
            tc.tile_pool(name="ps_misc", bufs=3, space="PSUM") as ps_misc,
            tc.tile_pool(name="dram", bufs=1, space="DRAM") as dram,
        ):
            # ---------------- constants + small loads ----------------
            ones_row = small.tile([1, 128], F32, tag="c0")
            ones_col = small.tile([128, 1], F32, tag="c1")
            one_1x1 = small.tile([1, 1], F32, tag="c2")
            nc.vector.memset(ones_row[:], 1.0)
            nc.vector.memset(ones_col[:], 1.0)
            nc.vector.memset(one_1x1[:], 1.0)

            x_cols = small.tile([128, KT], F32, tag="xc")
            an_cols = small.tile([128, KT], F32, tag="anc")
            fn_cols = small.tile([128, KT], F32, tag="fnc")
            nc.gpsimd.dma_start(x_cols[:], x_in.ap())
            nc.gpsimd.dma_start(an_cols[:], an_in.ap())
            nc.gpsimd.dma_start(fn_cols[:], fn_in.ap())

            cos4 = small.tile([1, QC], F32, tag="cos4")
            sin4 = small.tile([1, QC], F32, tag="sin4")
            nc.gpsimd.dma_start(cos4[:], cos_in.ap().rearrange("(p n) -> p n", p=1))
            nc.gpsimd.dma_start(sin4[:], sin_in.ap().rearrange("(p n) -> p n", p=1))
            eps_t = small.tile([1, 1], F32, tag="eps")
            nc.vector.memset(eps_t[:], EPS)
            cos4q = small.tile([1, QC], F32, tag="cos4q")
            sin4q = small.tile([1, QC], F32, tag="sin4q")
            nc.vector.tensor_scalar_mul(cos4q[:], cos4[:], SCALE)
            nc.vector.tensor_scalar_mul(sin4q[:], sin4[:], SCALE)

            def rmsnorm(x_t, norm_t, out_t):
                """out = x * norm * rsqrt(mean(x^2) + eps), all [128, KT] cols."""
                scr = work.tile([128, KT], F32, tag="rms_scr")
                ssq = work.tile([128, 1], F32, tag="rms_ssq")
                nc.vector.scalar_tensor_tensor(
                    out=scr[:], in0=x_t[:], scalar=1.0, in1=x_t[:],
                    op0=ALU.mult, op1=ALU.mult, accum_out=ssq[:],
                )
                tot = ps_misc.tile([1, 1], F32, tag="wide")
                nc.tensor.matmul(tot[:], ones_col[:], ssq[:], start=True, stop=True)
                rms = work.tile([1, 1], F32, tag="rms_rms")
                nc.scalar.activation(rms[:], tot[:], AF.Sqrt, bias=eps_t[:], scale=1.0 / H)
                rinv = work.tile([1, 1], F32, tag="rms_rinv")
                nc.vector.reciprocal(rinv[:], rms[:])
                rb_ps = ps_misc.tile([128, 1], F32, tag="wide")
                nc.tensor.matmul(rb_ps[:], ones_row[:], rinv[:], start=True, stop=True)
                rb_sb = work.tile([128, 1], F32, tag="rms_rb_sb")
                nc.vector.tensor_copy(rb_sb[:], rb_ps[:])
                scl = work.tile([128, KT], F32, tag="rms_scl")
                nc.scalar.activation(scl[:], x_t[:], AF.Copy, scale=rb_sb[:])
                nc.vector.tensor_mul(out_t[:], scl[:], norm_t[:])

            # ---------------- rmsnorm 1 ----------------
            h_cols = small.tile([128, KT], F32, tag="hc")
            rmsnorm(x_cols, an_cols, h_cols)

            # ---------------- q/k/v GEMV ----------------
            q_ps = ps_row.tile([1, QC], F32, tag="row")
            k_ps = ps_row.tile([1, QC], F32, tag="row")
            v_ps = ps_row.tile([1, QC], F32, tag="row")
            for g in range(4):
                wq_c = stream.tile([128, 8, 512], F32, tag="wstream")
                wk_c = stream.tile([128, 8, 512], F32, tag="wstream")
                wv_c = stream.tile([128, 8, 512], F32, tag="wstream")
                nc.sync.dma_start(wq_c[:], wq_v[g])
                nc.sync.dma_start(wk_c[:], wk_v[g])
                nc.sync.dma_start(wv_c[:], wv_v[g])
                for j in range(8):
                    kt = g * 8 + j
                    st, sp = (kt == 0), (kt == KT - 1)
                    lhs = h_cols[:, kt : kt + 1]
                    nc.tensor.matmul(q_ps[:], lhs, wq_c[:, j, :], start=st, stop=sp)
                    nc.tensor.matmul(k_ps[:], lhs, wk_c[:, j, :], start=st, stop=sp)
                    nc.tensor.matmul(v_ps[:], lhs, wv_c[:, j, :], start=st, stop=sp)

            # ---------------- RoPE ----------------
            def rope(src_ps, cos_t, sin_t, out_row, nm):
                rot = work.tile([1, HPC, 2, 64], F32, tag=f"rot_{nm}", name=f"rot{nm}")
                sv = src_ps[:].rearrange("p (h t d) -> p h t d", h=HPC, t=2)
                nc.scalar.activation(rot[:, :, 0, :], sv[:, :, 1, :], AF.Copy, scale=-1.0)
                nc.scalar.activation(rot[:, :, 1, :], sv[:, :, 0, :], AF.Copy, scale=1.0)
                t1 = work.tile([1, QC], F32, tag=f"t1_{nm}", name=f"t1{nm}")
                t2 = work.tile([1, QC], F32, tag=f"t2_{nm}", name=f"t2{nm}")
                nc.vector.tensor_mul(t1[:], src_ps[:], cos_t)
                nc.vector.tensor_mul(t2[:], rot[:].rearrange("p h t d -> p (h t d)"), sin_t)
                nc.vector.tensor_add(out_row[:], t1[:], t2[:])

            q_row = small.tile([1, QC], F32, tag="q_row")
            k_row = small.tile([1, QC], F32, tag="k_row")
            v_row = small.tile([1, QC], F32, tag="v_row")
            rope(q_ps, cos4q[:], sin4q[:], q_row, "q")  # q pre-scaled by 1/sqrt(hd)
            rope(k_ps, cos4[:], sin4[:], k_row, "k")
            nc.vector.tensor_copy(v_row[:], v_ps[:])

            # broadcast q across partitions via ones outer-product: [128, 512]
            qb_ps = ps_misc.tile([128, QC], F32, tag="qb_ps", bufs=1)
            nc.tensor.matmul(qb_ps[:], ones_row[:], q_row[:], start=True, stop=True)
            qb = small.tile([128, QC], F32, tag="qb")
            nc.vector.tensor_copy(qb[:], qb_ps[:])

            # ---------------- scores over K cache ----------------
            scores = [
                small.tile([128, KT], F32, tag=f"sc{h}", name=f"scores{h}")
                for h in range(HPC)
            ]
            for h in range(HPC):
                nc.vector.memset(scores[h][:], -1e30)
            scr = work.tile([128, 128], F32, tag="ttr_scr")

            n_kv_chunks = (n_tiles + 7) // 8
            for c in range(n_kv_chunks):
                kch = stream.tile([128, 8, QC], F32, tag="wstream")
                s_hi = min(8, n_tiles - c * 8)
                full = (c * 8 + s_hi) * 128 <= n_s
                n_full_s = s_hi if full else s_hi - 1
                if n_full_s > 0:
                    nc.sync.dma_start(kch[:, 0:n_full_s, :], kc_v[c][:, 0:n_full_s, :])
                if not full:
                    nc.sync.dma_start(kch[0:rem, s_hi - 1, :], kc_v[c][0:rem, s_hi - 1, :])
                if pos_tile // 8 == c:
                    nc.gpsimd.dma_start(
                        kch[pos_row : pos_row + 1, pos_tile % 8, :], k_row[:]
                    )
                for s in range(s_hi):
                    stt = c * 8 + s
                    w = 128 if (stt + 1) * 128 <= n_s else rem
                    for h in range(HPC):
                        nc.vector.scalar_tensor_tensor(
                            out=scr[0:w, :],
                            in0=kch[0:w, s, h * HD : (h + 1) * HD],
                            scalar=1.0,
                            in1=qb[0:w, h * HD : (h + 1) * HD],
                            op0=ALU.mult,
                            op1=ALU.mult,
                            accum_out=scores[h][0:w, stt : stt + 1],
                        )

            # ---------------- softmax (exact max via row-gather DMA) -------------
            maxes = work.tile([128, HPC], F32, tag="maxes")
            for h in range(HPC):
                nc.vector.reduce_max(maxes[:, h : h + 1], scores[h][:], axis=AX.X)
            gmax_all = work.tile([128, HPC], F32, tag="gmax_all")
            nc.gpsimd.partition_all_reduce(
                gmax_all[:], maxes[:], 128, bass_isa.ReduceOp.max
            )
            nmax_sb = work.tile([128, HPC], F32, tag="nmax_sb")
            nc.vector.tensor_scalar_mul(nmax_sb[:], gmax_all[:], -1.0)

            exps = [
                small.tile([128, KT], F32, tag=f"ex{h}", name=f"exps{h}")
                for h in range(HPC)
            ]
            sums = work.tile([128, HPC], F32, tag="sums")
            for h in range(HPC):
                nc.scalar.activation(
                    exps[h][:], scores[h][:], AF.Exp,
                    bias=nmax_sb[:, h : h + 1], scale=1.0,
                    accum_out=sums[:, h : h + 1],
                )
            tot4_ps = ps_misc.tile([1, HPC], F32, tag="wide")
            nc.tensor.matmul(tot4_ps[:], ones_col[:], sums[:], start=True, stop=True)
            tot4 = work.tile([1, HPC], F32, tag="tot4_sb")
            nc.vector.tensor_copy(tot4[:], tot4_ps[:])
            rec4 = work.tile([1, HPC], F32, tag="rec4")
            nc.vector.reciprocal(rec4[:], tot4[:])
            rb4_ps = ps_misc.tile([128, HPC], F32, tag="wide")
            nc.tensor.matmul(rb4_ps[:], ones_row[:], rec4[:], start=True, stop=True)
            rb4_sb = work.tile([128, HPC], F32, tag="rb4_sb")
            nc.vector.tensor_copy(rb4_sb[:], rb4_ps[:])

            # ---------------- o = softmax @ V (per-head row psum banks) ---------
            o_ps = [
                ps_row.tile([1, HD], F32, tag="row", name=f"ops{h}")
                for h in range(HPC)
            ]
            last_t = n_tiles - 1
            for c in range(n_kv_chunks):
                vch = stream.tile([128, 8, QC], F32, tag="wstream")
                s_hi = min(8, n_tiles - c * 8)
                full = (c * 8 + s_hi) * 128 <= n_s
                n_full_s = s_hi if full else s_hi - 1
                if n_full_s > 0:
                    nc.sync.dma_start(vch[:, 0:n_full_s, :], vc_v[c][:, 0:n_full_s, :])
                if not full:
                    nc.sync.dma_start(vch[0:rem, s_hi - 1, :], vc_v[c][0:rem, s_hi - 1, :])
                if pos_tile // 8 == c:
                    nc.gpsimd.dma_start(
                        vch[pos_row : pos_row + 1, pos_tile % 8, :], v_row[:]
                    )
                for s in range(s_hi):
                    stt = c * 8 + s
                    w = 128 if (stt + 1) * 128 <= n_s else rem
                    for h in range(HPC):
                        nc.tensor.matmul(
                            o_ps[h][:],
                            exps[h][0:w, stt : stt + 1],
                            vch[0:w, s, h * HD : (h + 1) * HD],
                            start=(stt == 0),
                            stop=(stt == last_t),
                        )

            o_row = work.tile([1, QC], F32, tag="o_row")
            for h in range(HPC):
                nc.vector.tensor_copy(o_row[0:1, h * HD : (h + 1) * HD], o_ps[h][:])
            o_dram = dram.tile([QC], F32, tag="o_dram")
            nc.sync.dma_start(o_dram[:].rearrange("(p n) -> p n", p=1), o_row[:])
            o_cols = work.tile([128, HPC], F32, tag="o_cols")
            nc.gpsimd.dma_start(o_cols[:], o_dram[:].rearrange("(t p) -> p t", p=128))
            o_sb = work.tile([128, HPC], F32, tag="o_sb")
            nc.vector.tensor_mul(o_sb[:], o_cols[:], rb4_sb[:])

            # ---------------- attn partial rows = o @ wo ----------------
            wo_chunks = []
            for c in range(4):
                wo_c = stream.tile([128, H], F32, tag="wstream", name=f"wo{c}")
                nc.sync.dma_start(wo_c[:], wo_v[c])
                wo_chunks.append(wo_c)
            xattn_row = work.tile([1, H], F32, tag="xattn_row")
            for i in range(8):
                wo_ps = ps_row.tile([1, 512], F32, tag="row", name=f"wops{i}")
                for c in range(4):
                    nc.tensor.matmul(
                        wo_ps[:],
                        o_sb[:, c : c + 1],
                        wo_chunks[c][:, i * 512 : (i + 1) * 512],
                        start=(c == 0),
                        stop=(c == 3),
                    )
                nc.scalar.copy(xattn_row[0:1, i * 512 : (i + 1) * 512], wo_ps[:])

            # ---------------- AllReduce attn partial (row layout) -------------
            ar_in = dram.tile([H], F32)
            ar_out = dram.tile([H], F32)
            nc.sync.dma_start(ar_in[:].rearrange("(p n) -> p n", p=1), xattn_row[:])
            nc.gpsimd.collective_compute(
                "AllReduce",
                ALU.add,
                replica_groups=[list(range(CORES))],
                ins=[ar_in[:].opt()],
                outs=[ar_out[:].opt()],
            )
            attnsum = small.tile([128, KT], F32, tag="attnsum")
            nc.gpsimd.dma_start(attnsum[:], ar_out[:].rearrange("(t p) -> p t", p=128))

            # ---------------- residual + rmsnorm 2 ----------------
            xnew = small.tile([128, KT], F32, tag="xnew")
            nc.vector.tensor_add(xnew[:], x_cols[:], attnsum[:])
            nc.sync.dma_start(xnew_out.ap(), xnew[:])
            h2 = small.tile([128, KT], F32, tag="h2")
            rmsnorm(xnew, fn_cols, h2)

            # ---------------- ff1 GEMV ----------------
            f1_ps = [
                ps_row.tile([1, min(512, FFC - 512 * i)], F32, tag="row", name=f"f1ps{i}")
                for i in range(NFF1T)
            ]
            for c in range(16):
                f1c = stream.tile([128, 2, FFC], F32, tag="wstream")
                nc.sync.dma_start(f1c[:], ff1_v[c])
                for j in range(2):
                    kt = c * 2 + j
                    st, sp = (kt == 0), (kt == KT - 1)
                    lhs = h2[:, kt : kt + 1]
                    for i in range(NFF1T):
                        lo, hi = i * 512, min((i + 1) * 512, FFC)
                        nc.tensor.matmul(
                            f1_ps[i][:], lhs, f1c[:, j, lo:hi], start=st, stop=sp
                        )
            ff1row = work.tile([1, FFC], F32, tag="ff1row")
            for i in range(NFF1T):
                lo, hi = i * 512, min((i + 1) * 512, FFC)
                nc.scalar.copy(ff1row[0:1, lo:hi], f1_ps[i][:])

            # ---------------- silu (cols layout via row->col DMA) -------------
            pre_cols = work.tile([128, FFKT], F32, tag="pre_cols")
            wlast = FFC % 128
            nfull = FFKT - 1 if wlast else FFKT
            ff1_dram = dram.tile([FFC], F32, tag="ff1_dram")
            nc.sync.dma_start(ff1_dram[:].rearrange("(p n) -> p n", p=1), ff1row[:])
            nc.gpsimd.dma_start(
                pre_cols[:, 0:nfull],
                bass.AP(tensor=ff1_dram[:].tensor, offset=ff1_dram[:].offset,
                        ap=[[1, 128], [128, nfull]]),
            )
            if wlast:
                nc.gpsimd.dma_start(
                    pre_cols[0:wlast, nfull : nfull + 1],
                    bass.AP(tensor=ff1_dram[:].tensor,
                            offset=ff1_dram[:].offset + nfull * 128,
                            ap=[[1, wlast], [128, 1]]),
                )
            sg_sb = work.tile([128, FFKT], F32, tag="sg")
            silu = work.tile([128, FFKT], F32, tag="silu")
            if wlast:
                nc.scalar.activation(sg_sb[0:wlast, :], pre_cols[0:wlast, :], AF.Sigmoid)
                nc.vector.tensor_mul(
                    silu[0:wlast, :], sg_sb[0:wlast, :], pre_cols[0:wlast, :]
                )
                nc.scalar.activation(
                    sg_sb[wlast:128, 0:nfull], pre_cols[wlast:128, 0:nfull], AF.Sigmoid
                )
                nc.vector.tensor_mul(
                    silu[wlast:128, 0:nfull],
                    sg_sb[wlast:128, 0:nfull],
                    pre_cols[wlast:128, 0:nfull],
                )
            else:
                nc.scalar.activation(sg_sb[:], pre_cols[:], AF.Sigmoid)
                nc.vector.tensor_mul(silu[:], sg_sb[:], pre_cols[:])

            # ---------------- ff2 GEMV (rows, SBUF accumulation) -------------
            ffacc = work.tile([1, H], F32, tag="bigrow")
            nc.vector.memset(ffacc[:], 0.0)
            for it in range(FFKT):
                w = min(128, FFC - it * 128)
                f2c = stream.tile([128, H], F32, tag="wstream")
                nc.sync.dma_start(f2c[0:w, :], ff2_v[it * 128 : it * 128 + w, :])
                lhs = silu[0:w, it : it + 1]
                for i in range(8):
                    tmp = ps_row.tile([1, 512], F32, tag="row", name=f"f2tmp_{it}_{i}")
                    nc.tensor.matmul(
                        tmp[:], lhs, f2c[0:w, i * 512 : (i + 1) * 512],
                        start=True, stop=True,
                    )
                    sl = ffacc[0:1, i * 512 : (i + 1) * 512]
                    nc.vector.tensor_add(sl, sl, tmp[:])
            nc.sync.dma_start(ff_out.ap().rearrange("(p n) -> p n", p=1), ffacc[:])

    nc.compile()
    _BUILD_CACHE[pos] = nc
    return nc


def _shard(inputs, pos):
    x = np.ascontiguousarray(np.asarray(inputs["x"], dtype=np.float32))
    an = np.ascontiguousarray(np.asarray(inputs["attn_norm"], dtype=np.float32))
    fn = np.ascontiguousarray(np.asarray(inputs["ffn_norm"], dtype=np.float32))
    cos_r = np.ascontiguousarray(np.asarray(inputs["cos_cache"], dtype=np.float32)[pos])
    sin_r = np.ascontiguousarray(np.asarray(inputs["sin_cache"], dtype=np.float32)[pos])
    wq = np.asarray(inputs["w_q"], dtype=np.float32)
    wk = np.asarray(inputs["w_k"], dtype=np.float32)
    wv = np.asarray(inputs["w_v"], dtype=np.float32)
    wo = np.asarray(inputs["w_o"], dtype=np.float32)
    kc = np.asarray(inputs["k_cache"], dtype=np.float32)
    vc = np.asarray(inputs["v_cache"], dtype=np.float32)
    ff1 = np.asarray(inputs["w_ff1"], dtype=np.float32)
    ff2 = np.asarray(inputs["w_ff2"], dtype=np.float32)

    def cols(v):
        return np.ascontiguousarray(v.reshape(KT, 128).T)

    x_c, an_c, fn_c = cols(x), cols(an), cols(fn)
    cos4 = np.ascontiguousarray(np.tile(cos_r, HPC))
    sin4 = np.ascontiguousarray(np.tile(sin_r, HPC))
    in_maps = []
    for c in range(CORES):
        qlo, qhi = c * QC, (c + 1) * QC
        flo, fhi = c * FFC, (c + 1) * FFC
        hlo, hhi = c * HPC, (c + 1) * HPC
        in_maps.append(
            {
                "x_cols": x_c,
                "an_cols": an_c,
                "fn_cols": fn_c,
                "cos4": cos4,
                "sin4": sin4,
                "wq": np.ascontiguousarray(wq[:, qlo:qhi]),
                "wk": np.ascontiguousarray(wk[:, qlo:qhi]),
                "wv": np.ascontiguousarray(wv[:, qlo:qhi]),
                "wo": np.ascontiguousarray(wo[qlo:qhi, :]),
                "kc": np.ascontiguousarray(kc[:, hlo:hhi, :]),
                "vc": np.ascontiguousarray(vc[:, hlo:hhi, :]),
                "ff1": np.ascontiguousarray(ff1[:, flo:fhi]),
                "ff2": np.ascontiguousarray(ff2[flo:fhi, :]),
            }
        )
    return in_maps


def _assemble(results):
    xnew_cols = results[0]["xnew_out"]  # [128, 32], element (p,t) = vec[t*128+p]
    xnew = np.ascontiguousarray(xnew_cols.T).reshape(-1)
    ff = np.sum(
        np.stack([results[c]["ff_out"] for c in range(CORES)]), axis=0,
        dtype=np.float32,
    )
    return (xnew + ff).astype(np.float32)


def run(inputs, trace=False):
    pos = int(inputs["pos"])
    nc = _build(pos)
    in_maps = _shard(inputs, pos)
    res = run_bass_kernel_spmd(nc, in_maps, core_ids=list(range(CORES)), trace=trace)
    return _assemble(res.results), res


def kernel(**inputs) -> np.ndarray:
    out, _ = run(inputs, trace=False)
    return out



# revision 9
# speedup vs baseline: 2.4729x; 2.4729x over previous
"""Llama block (single-token decode) on 8 TRN2 NeuronCores, tensor-parallel.

Sharding (per core c of 8):
  - heads 4c..4c+3: w_q/w_k/w_v column shards [4096, 512], KV cache [4096, 4, 128]
  - w_o row shard [512, 4096] -> partial attn output, AllReduce'd on device
  - w_ff1 column shard [4096, 1376->1408pad], w_ff2 row shard [1376->1408pad, 4096]
  - per-core FFN partials summed on host (row-sharded output unshard)

Perf design vs the f32 baseline:
  - All large tensors cast to bf16 ON HOST and repacked tile-major so every
    streaming DMA is a contiguous ~1 MB transfer (descriptor-efficient).
  - One HWDGE queue (nc.sync) carries the 43 stream chunks in exact
    consumption order: wq, wk, kc, wv, vc, wo, ff1, ff2.  All small or
    dependency-carrying DMAs ride SWDGE (nc.gpsimd) so the stream never
    blocks behind the AllReduce.
  - Scores are batched DVE ops (one mult + one segmented reduce per KV chunk)
    instead of 128 tiny scalar_tensor_tensor calls.
  - o = softmax@V is computed weight-stationary (V tile as lhsT) so the
    result lands directly in [128d, 4h] layout - no transpose step.
  - ff2 accumulates in PSUM across all 11 k-tiles (two 4-bank passes over
    resident chunks), no DVE row adds.
  - Row->cols layout moves use PE transposes (no DRAM round-trips).
  - ACT function tables (sqrt/exp/sigmoid) are preloaded at t=0.
  - AllReduce payload is bf16 (8 KB) and overlaps the ff weight stream.

On-chip layout convention: a length-4096 vector is held as [128, 32] "cols"
(element (p, t) = vec[t*128 + p]) so vector tiles feed matmul lhsT directly.
"""

import math
import sys

sys.path.insert(0, "/opt/trn_rl_repo")

import numpy as np
import ml_dtypes

import concourse.bass as bass
import concourse.tile as tile
from concourse import bacc, mybir
from concourse.bass_utils import run_bass_kernel_spmd
from concourse import bass_isa

F32 = mybir.dt.float32
BF16 = mybir.dt.bfloat16
AF = mybir.ActivationFunctionType
ALU = mybir.AluOpType
AX = mybir.AxisListType

H = 4096
NH = 32
HD = 128
INTERM = 11008
EPS = 1e-6
CORES = 8
HPC = NH // CORES  # 4 heads per core
QC = HPC * HD  # 512 qkv cols per core
FFC = INTERM // CORES  # 1376 ff cols per core
FFCP = 1408  # padded to 11 * 128
FFKT = FFCP // 128  # 11 contraction tiles for ff2
KT = H // 128  # 32 contraction tiles
SCALE = 1.0 / math.sqrt(HD)
NP_BF16 = ml_dtypes.bfloat16

_BUILD_CACHE = {}


def _build(pos: int):
    if pos in _BUILD_CACHE:
        return _BUILD_CACHE[pos]

    n_s = pos + 1
    n_tiles = (n_s + 127) // 128  # s-tiles to attend over
    rem = n_s - (n_tiles - 1) * 128  # rows in last s-tile (1..128)
    pos_tile = pos // 128
    pos_row = pos % 128
    n_kv_chunks = (n_tiles + 7) // 8

    nc = bacc.Bacc("TRN2", target_bir_lowering=False, debug=False, num_devices=CORES)

    x_in = nc.dram_tensor("x_cols", [128, KT], F32, kind="ExternalInput")
    an_in = nc.dram_tensor("an_cols", [128, KT], F32, kind="ExternalInput")
    fn_in = nc.dram_tensor("fn_cols", [128, KT], F32, kind="ExternalInput")
    rope_in = nc.dram_tensor("rope_tbl", [4, QC], F32, kind="ExternalInput")
    eye32_in = nc.dram_tensor("eye32", [32, 32], F32, kind="ExternalInput")
    wq_in = nc.dram_tensor("wq", [4, 128, 8, QC], BF16, kind="ExternalInput")
    wk_in = nc.dram_tensor("wk", [4, 128, 8, QC], BF16, kind="ExternalInput")
    wv_in = nc.dram_tensor("wv", [4, 128, 8, QC], BF16, kind="ExternalInput")
    wo_in = nc.dram_tensor("wo", [4, 128, H], BF16, kind="ExternalInput")
    kc_in = nc.dram_tensor("kc", [4, 128, 8, QC], BF16, kind="ExternalInput")
    vc_in = nc.dram_tensor("vc", [4, 128, 8, QC], BF16, kind="ExternalInput")
    ff1_in = nc.dram_tensor("ff1", [8, 128, 4, FFCP], BF16, kind="ExternalInput")
    ff2_in = nc.dram_tensor("ff2", [FFKT, 128, H], BF16, kind="ExternalInput")

    xnew_out = nc.dram_tensor("xnew_out", [128, KT], F32, kind="ExternalOutput")
    ff_out = nc.dram_tensor("ff_out", [H], F32, kind="ExternalOutput")

    with tile.TileContext(nc) as tc:
        with (
            tc.tile_pool(name="stream", bufs=12) as stream,
            tc.tile_pool(name="prodp", bufs=1) as prodp,
            tc.tile_pool(name="small", bufs=1) as small,
            tc.tile_pool(name="work", bufs=1) as work,
            tc.tile_pool(name="ps_row", bufs=4, space="PSUM") as ps_row,
            tc.tile_pool(name="ps_qb", bufs=1, space="PSUM") as ps_qb,
            tc.tile_pool(name="ps_small", bufs=2, space="PSUM") as ps_small,
            tc.tile_pool(name="dram", bufs=1, space="DRAM") as dram,
        ):
            # ---------------- constants + small loads ----------------
            ones_row = small.tile([1, 128], F32, tag="c0")
            ones_col = small.tile([128, 1], F32, tag="c1")
            nc.vector.memset(ones_row[:], 1.0)
            nc.vector.memset(ones_col[:], 1.0)
            ones_row_bf = small.tile([1, 128], BF16, tag="c3")
            nc.vector.memset(ones_row_bf[:], 1.0)
            eps_t = small.tile([1, 1], F32, tag="eps")
            nc.vector.memset(eps_t[:], EPS)

            # preload ACT tables (sqrt/exp/sigmoid) so the first real use
            # doesn't pay the table-load latency on the critical path
            warm = work.tile([1, 1], F32, tag="warm")
            nc.scalar.activation(warm[:], eps_t[:], AF.Sqrt)
            nc.scalar.activation(warm[:], eps_t[:], AF.Exp)
            nc.scalar.activation(warm[:], eps_t[:], AF.Sigmoid)

            x_cols = small.tile([128, KT], F32, tag="xc")
            an_cols = small.tile([128, KT], F32, tag="anc")
            fn_cols = small.tile([128, KT], F32, tag="fnc")
            rope_rows = [
                small.tile([1, QC], F32, tag=f"rope{r}", name=f"rope{r}")
                for r in range(4)
            ]
            eye32 = small.tile([32, 32], F32, tag="eye32")
            nc.gpsimd.dma_start(x_cols[:], x_in.ap())
            nc.gpsimd.dma_start(an_cols[:], an_in.ap())
            nc.gpsimd.dma_start(fn_cols[:], fn_in.ap())
            for r in range(4):
                nc.gpsimd.dma_start(rope_rows[r][:], rope_in.ap()[r : r + 1, :])
            nc.gpsimd.dma_start(eye32[:], eye32_in.ap())

            def rmsnorm(x_t, norm_t, out_t, nm):
                """out = x * norm * rsqrt(mean(x^2) + eps), [128, KT] cols."""
                scr = work.tile([128, KT], F32, tag=f"rms_scr{nm}")
                ssq = work.tile([128, 1], F32, tag=f"rms_ssq{nm}")
                nc.vector.scalar_tensor_tensor(
                    out=scr[:], in0=x_t[:], scalar=1.0, in1=x_t[:],
                    op0=ALU.mult, op1=ALU.mult, accum_out=ssq[:],
                )
                tot = ps_small.tile([1, 1], F32, tag="sm", name=f"rmst{nm}")
                nc.tensor.matmul(tot[:], ones_col[:], ssq[:], start=True, stop=True)
                rms = work.tile([1, 1], F32, tag=f"rms_rms{nm}")
                nc.scalar.activation(rms[:], tot[:], AF.Sqrt, bias=eps_t[:], scale=1.0 / H)
                rinv = work.tile([1, 1], F32, tag=f"rms_rinv{nm}")
                nc.vector.reciprocal(rinv[:], rms[:])
                rb_ps = ps_small.tile([128, 1], F32, tag="sm", name=f"rmsb{nm}")
                nc.tensor.matmul(rb_ps[:], ones_row[:], rinv[:], start=True, stop=True)
                rb_sb = work.tile([128, 1], F32, tag=f"rms_rb{nm}")
                nc.vector.tensor_copy(rb_sb[:], rb_ps[:])
                scl = work.tile([128, KT], F32, tag=f"rms_scl{nm}")
                nc.scalar.activation(scl[:], x_t[:], AF.Copy, scale=rb_sb[:])
                nc.vector.tensor_mul(out_t[:], scl[:], norm_t[:])

            # ---------------- rmsnorm 1 (h in bf16) ----------------
            h_bf = small.tile([128, KT], BF16, tag="hbf")
            rmsnorm(x_cols, an_cols, h_bf, "1")

            # ---------------- stream DMAs (one HWDGE queue, usage order) ------
            wq_c = [None] * 4
            wk_c = [None] * 4
            wv_c = [None] * 4
            for g in range(4):
                wq_c[g] = stream.tile([128, 8, QC], BF16, tag="stream", name=f"wq{g}")
                nc.sync.dma_start(wq_c[g][:], wq_in.ap()[g])
            for g in range(4):
                wk_c[g] = stream.tile([128, 8, QC], BF16, tag="stream", name=f"wk{g}")
                nc.sync.dma_start(wk_c[g][:], wk_in.ap()[g])
            # K-cache chunks right after wk so scores can start early
            kch = [None] * n_kv_chunks
            for c in range(n_kv_chunks):
                kch[c] = stream.tile([128, 8, QC], BF16, tag="stream", name=f"kc{c}")
                s_hi = min(8, n_tiles - c * 8)
                full = (c * 8 + s_hi) * 128 <= n_s
                n_full_s = s_hi if full else s_hi - 1
                if n_full_s > 0:
                    nc.sync.dma_start(kch[c][:, 0:n_full_s, :], kc_in.ap()[c][:, 0:n_full_s, :])
                if not full:
                    nc.sync.dma_start(kch[c][0:rem, s_hi - 1, :], kc_in.ap()[c][0:rem, s_hi - 1, :])
            for g in range(4):
                wv_c[g] = stream.tile([128, 8, QC], BF16, tag="stream", name=f"wv{g}")
                nc.sync.dma_start(wv_c[g][:], wv_in.ap()[g])
            vch = [None] * n_kv_chunks
            for c in range(n_kv_chunks):
                vch[c] = stream.tile([128, 8, QC], BF16, tag="stream", name=f"vc{c}")
                s_hi = min(8, n_tiles - c * 8)
                full = (c * 8 + s_hi) * 128 <= n_s
                n_full_s = s_hi if full else s_hi - 1
                if n_full_s > 0:
                    nc.sync.dma_start(vch[c][:, 0:n_full_s, :], vc_in.ap()[c][:, 0:n_full_s, :])
                if not full:
                    nc.sync.dma_start(vch[c][0:rem, s_hi - 1, :], vc_in.ap()[c][0:rem, s_hi - 1, :])
            wo_c = [None] * 4
            for g in range(4):
                wo_c[g] = stream.tile([128, H], BF16, tag="stream", name=f"wo{g}")
                nc.sync.dma_start(wo_c[g][:], wo_in.ap()[g])
            ff1_c = [None] * 8
            for g in range(8):
                ff1_c[g] = stream.tile([128, 4, FFCP], BF16, tag="stream", name=f"f1{g}")
                nc.sync.dma_start(ff1_c[g][:], ff1_in.ap()[g])
            ff2_c = [None] * FFKT
            for g in range(FFKT):
                ff2_c[g] = stream.tile([128, H], BF16, tag="stream", name=f"f2{g}")
                nc.sync.dma_start(ff2_c[g][:], ff2_in.ap()[g])

            # ---------------- q/k/v GEMV (3 psum rows) ----------------
            q_ps = ps_row.tile([1, QC], F32, tag="row", name="qps")
            k_ps = ps_row.tile([1, QC], F32, tag="row", name="kps")
            v_ps = ps_row.tile([1, QC], F32, tag="row", name="vps")
            for g in range(4):
                for j in range(8):
                    kt = g * 8 + j
                    st, sp = (kt == 0), (kt == KT - 1)
                    lhs = h_bf[:, kt : kt + 1]
                    nc.tensor.matmul(q_ps[:], lhs, wq_c[g][:, j, :], start=st, stop=sp)
                    nc.tensor.matmul(k_ps[:], lhs, wk_c[g][:, j, :], start=st, stop=sp)
                    nc.tensor.matmul(v_ps[:], lhs, wv_c[g][:, j, :], start=st, stop=sp)

            # ---------------- RoPE (f32 in, bf16 rows out) ----------------
            # rope_tbl rows: 0=cos, 1=sin, 2=cos*SCALE, 3=sin*SCALE
            def rope(src, cos_t, sin_t, out_row, nm):
                sv = src.rearrange("p (h t d) -> p h t d", h=HPC, t=2)
                rot = work.tile([1, HPC, 2, 64], F32, tag=f"rot_{nm}", name=f"rot{nm}")
                nc.scalar.activation(rot[:, :, 0, :], sv[:, :, 1, :], AF.Copy, scale=-1.0)
                nc.scalar.activation(rot[:, :, 1, :], sv[:, :, 0, :], AF.Copy, scale=1.0)
                t1 = work.tile([1, QC], F32, tag="t1", name=f"t1{nm}")
                t2 = work.tile([1, QC], F32, tag="t2", name=f"t2{nm}")
                nc.vector.tensor_mul(t1[:], src, cos_t)
                nc.vector.tensor_mul(t2[:], rot[:].rearrange("p h t d -> p (h t d)"), sin_t)
                nc.vector.tensor_add(out_row[:], t1[:], t2[:])

            q_row = small.tile([1, QC], BF16, tag="q_row")
            k_row = small.tile([1, QC], BF16, tag="k_row")
            v_row = small.tile([1, QC], BF16, tag="v_row")
            rope(q_ps[:], rope_rows[2][:], rope_rows[3][:], q_row, "q")  # pre-scaled
            rope(k_ps[:], rope_rows[0][:], rope_rows[1][:], k_row, "k")
            nc.vector.tensor_copy(v_row[:], v_ps[:])

            # broadcast q across partitions: qb[p, :] = q_row for every p
            qb_ps = ps_qb.tile([128, QC], F32, tag="qbps")
            nc.tensor.matmul(qb_ps[:], ones_row_bf[:], q_row[:], start=True, stop=True)
            qb = small.tile([128, QC], BF16, tag="qb")
            nc.vector.tensor_copy(qb[:], qb_ps[:])

            # insert current token's k/v into the streamed cache chunks
            nc.gpsimd.dma_start(
                kch[pos_tile // 8][pos_row : pos_row + 1, pos_tile % 8, :], k_row[:]
            )
            nc.gpsimd.dma_start(
                vch[pos_tile // 8][pos_row : pos_row + 1, pos_tile % 8, :], v_row[:]
            )

            # ---------------- scores over K cache (batched DVE) --------------
            # scores_all[p, st, h] = q[h] . k[st*128+p, h] (pre-scaled via q)
            scores_all = small.tile([128, KT, HPC], F32, tag="scores")
            nc.vector.memset(scores_all[:], -1e30)
            p_stride = qb.ap[0][0]
            qb_bcast = bass.AP(
                tensor=qb.tensor,
                offset=qb.offset,
                ap=[[p_stride, 128], [0, 8], [1, QC]],
            )
            for c in range(n_kv_chunks):
                s_hi = min(8, n_tiles - c * 8)
                full = (c * 8 + s_hi) * 128 <= n_s and s_hi == 8
                if full:
                    prod = prodp.tile([128, 8, QC], BF16, tag="prod")
                    nc.vector.tensor_mul(prod[:], kch[c][:], qb_bcast)
                    nc.vector.reduce_sum(
                        scores_all[:, c * 8 : (c + 1) * 8, :],
                        prod[:].rearrange("p s (h d) -> p s h d", h=HPC),
                        axis=AX.X,
                    )
                else:
                    sp_stride = scores_all.ap[0][0]
                    for s in range(s_hi):
                        stt = c * 8 + s
                        w = 128 if (stt + 1) * 128 <= n_s else rem
                        scr = work.tile([128, 128], F32, tag="ttr_scr")
                        for hh in range(HPC):
                            acc_ap = bass.AP(
                                tensor=scores_all.tensor,
                                offset=scores_all.offset + stt * HPC + hh,
                                ap=[[sp_stride, w], [1, 1]],
                            )
                            nc.vector.scalar_tensor_tensor(
                                out=scr[0:w, :],
                                in0=kch[c][0:w, s, hh * HD : (hh + 1) * HD],
                                scalar=1.0,
                                in1=qb[0:w, hh * HD : (hh + 1) * HD],
                                op0=ALU.mult,
                                op1=ALU.mult,
                                accum_out=acc_ap,
                            )

            # ---------------- softmax ----------------
            maxes = work.tile([128, HPC], F32, tag="maxes")
            sp_stride = scores_all.ap[0][0]
            sc_hst = bass.AP(
                tensor=scores_all.tensor,
                offset=scores_all.offset,
                ap=[[sp_stride, 128], [1, HPC], [HPC, KT]],
            )
            nc.vector.tensor_reduce(maxes[:], sc_hst, axis=AX.X, op=ALU.max)
            gmax = work.tile([128, HPC], F32, tag="gmax")
            nc.gpsimd.partition_all_reduce(gmax[:], maxes[:], 128, bass_isa.ReduceOp.max)
            nmax = work.tile([128, HPC], F32, tag="nmax")
            nc.vector.tensor_scalar_mul(nmax[:], gmax[:], -1.0)

            exps = [
                small.tile([128, KT], BF16, tag=f"ex{hh}", name=f"exps{hh}")
                for hh in range(HPC)
            ]
            sums = work.tile([128, HPC], F32, tag="sums")
            for hh in range(HPC):
                nc.scalar.activation(
                    exps[hh][:], scores_all[:, :, hh], AF.Exp,
                    bias=nmax[:, hh : hh + 1], scale=1.0,
                    accum_out=sums[:, hh : hh + 1],
                )
            tot4_ps = ps_small.tile([1, HPC], F32, tag="sm", name="tot4")
            nc.tensor.matmul(tot4_ps[:], ones_col[:], sums[:], start=True, stop=True)
            tot4 = work.tile([1, HPC], F32, tag="tot4_sb")
            nc.vector.tensor_copy(tot4[:], tot4_ps[:])
            rec4 = work.tile([1, HPC], F32, tag="rec4")
            nc.vector.reciprocal(rec4[:], tot4[:])
            # broadcast 1/sum to all partitions: rb4b[p, h] = rec4[h]
            rb4b_ps = ps_small.tile([128, HPC], F32, tag="sm", name="rb4b")
            nc.tensor.matmul(rb4b_ps[:], ones_row[:], rec4[:], start=True, stop=True)
            rb4b = work.tile([128, HPC], F32, tag="rb4b_sb")
            nc.vector.tensor_copy(rb4b[:], rb4b_ps[:])

            # ---------------- o = softmax @ V, weight-stationary -------------
            # oT_ps[d, h] accumulates sum_s exps[h][s] * v[s, h*HD+d]
            oT_ps = ps_small.tile([128, HPC], F32, tag="sm", name="oTps")
            last_t = n_tiles - 1
            for c in range(n_kv_chunks):
                s_hi = min(8, n_tiles - c * 8)
                for s in range(s_hi):
                    stt = c * 8 + s
                    w = 128 if (stt + 1) * 128 <= n_s else rem
                    for hh in range(HPC):
                        nc.tensor.matmul(
                            oT_ps[:, hh : hh + 1],
                            vch[c][0:w, s, hh * HD : (hh + 1) * HD],
                            exps[hh][0:w, stt : stt + 1],
                            start=(stt == 0),
                            stop=(stt == last_t),
                        )
            oT = work.tile([128, HPC], BF16, tag="oT_sb")
            nc.vector.tensor_mul(oT[:], oT_ps[:], rb4b[:])

            # ---------------- attn partial row = o @ wo ----------------
            xattn = work.tile([1, H], BF16, tag="xattn")
            for i in range(8):
                wo_ps = ps_row.tile([1, 512], F32, tag="row", name=f"wops{i}")
                for c in range(4):
                    nc.tensor.matmul(
                        wo_ps[:],
                        oT[:, c : c + 1],
                        wo_c[c][:, i * 512 : (i + 1) * 512],
                        start=(c == 0),
                        stop=(c == 3),
                    )
                nc.scalar.copy(xattn[0:1, i * 512 : (i + 1) * 512], wo_ps[:])

            # ---------------- AllReduce attn partial (bf16) -------------
            ar_in = dram.tile([H], BF16)
            ar_out = dram.tile([H], BF16)
            nc.gpsimd.dma_start(ar_in[:].rearrange("(p n) -> p n", p=1), xattn[:])
            nc.gpsimd.collective_compute(
                "AllReduce",
                ALU.add,
                replica_groups=[list(range(CORES))],
                ins=[ar_in[:].opt()],
                outs=[ar_out[:].opt()],
            )
            # rows [32,128] -> PE transpose -> cols [128,32]
            ar_rows = work.tile([32, 128], F32, tag="ar_rows")
            nc.gpsimd.dma_start(ar_rows[:], ar_out[:].rearrange("(t p) -> t p", p=128))
            at_ps = ps_small.tile([128, 32], F32, tag="sm", name="atps")
            nc.tensor.transpose(at_ps[:], ar_rows[:], eye32[:])
            attn_cols = work.tile([128, KT], F32, tag="attn_cols")
            nc.vector.tensor_copy(attn_cols[:], at_ps[:])

            # ---------------- residual + rmsnorm 2 ----------------
            xnew = small.tile([128, KT], F32, tag="xnew")
            nc.vector.tensor_add(xnew[:], x_cols[:], attn_cols[:])
            nc.gpsimd.dma_start(xnew_out.ap(), xnew[:])
            h2_bf = small.tile([128, KT], BF16, tag="h2bf")
            rmsnorm(xnew, fn_cols, h2_bf, "2")

            # ---------------- ff1 GEMV (3 psum rows: 512+512+384) -------------
            widths = [(0, 512), (512, 1024), (1024, FFCP)]
            f1_ps = [
                ps_row.tile([1, hi - lo], F32, tag="row", name=f"f1ps{i}")
                for i, (lo, hi) in enumerate(widths)
            ]
            for g in range(8):
                for j in range(4):
                    kt = g * 4 + j
                    st, sp = (kt == 0), (kt == KT - 1)
                    lhs = h2_bf[:, kt : kt + 1]
                    for i, (lo, hi) in enumerate(widths):
                        nc.tensor.matmul(
                            f1_ps[i][:], lhs, ff1_c[g][:, j, lo:hi], start=st, stop=sp
                        )

            # ---------------- silu + PE transpose to cols [128, 11] ----------
            silu_row = work.tile([1, FFCP], F32, tag="silu")
            for i, (lo, hi) in enumerate(widths):
                sg = work.tile([1, hi - lo], F32, tag=f"sg{i}")
                nc.scalar.activation(sg[:], f1_ps[i][:], AF.Sigmoid)
                nc.vector.tensor_mul(silu_row[0:1, lo:hi], sg[:], f1_ps[i][:])
            siluT_ps = ps_small.tile([128, FFKT], F32, tag="sm", name="siluTps")
            for t in range(FFKT):
                nc.tensor.transpose(
                    siluT_ps[:, t : t + 1],
                    silu_row[0:1, t * 128 : (t + 1) * 128],
                    ones_row[0:1, 0:1],
                )
            silu_cols = work.tile([128, FFKT], BF16, tag="silu_cols")
            nc.vector.tensor_copy(silu_cols[:], siluT_ps[:])

            # ---------------- ff2 GEMV (PSUM accumulate, 2 passes x 4 strips) -
            ffrow = work.tile([1, H], F32, tag="ffrow")
            for half in range(2):
                f2_ps = [
                    ps_row.tile([1, 512], F32, tag="row", name=f"f2ps{half}_{i}")
                    for i in range(4)
                ]
                for kt in range(FFKT):
                    lhs = silu_cols[:, kt : kt + 1]
                    for i in range(4):
                        strip = half * 4 + i
                        nc.tensor.matmul(
                            f2_ps[i][:],
                            lhs,
                            ff2_c[kt][:, strip * 512 : (strip + 1) * 512],
                            start=(kt == 0),
                            stop=(kt == FFKT - 1),
                        )
                for i in range(4):
                    strip = half * 4 + i
                    nc.scalar.copy(
                        ffrow[0:1, strip * 512 : (strip + 1) * 512], f2_ps[i][:]
                    )
            nc.gpsimd.dma_start(ff_out.ap().rearrange("(p n) -> p n", p=1), ffrow[:])

    nc.compile()
    _BUILD_CACHE[pos] = nc
    return nc


def _pack_k8(w):
    """[4096, N] -> [4, 128, 8, N] tile-major (chunk g, partition p, j, n)."""
    n = w.shape[1]
    return np.ascontiguousarray(
        w.reshape(4, 8, 128, n).transpose(0, 2, 1, 3)
    )


def _shard(inputs, pos):
    f32 = np.float32
    x = np.asarray(inputs["x"], f32)
    an = np.asarray(inputs["attn_norm"], f32)
    fn = np.asarray(inputs["ffn_norm"], f32)
    cos_r = np.asarray(inputs["cos_cache"], f32)[pos]
    sin_r = np.asarray(inputs["sin_cache"], f32)[pos]
    wq = np.asarray(inputs["w_q"], f32)
    wk = np.asarray(inputs["w_k"], f32)
    wv = np.asarray(inputs["w_v"], f32)
    wo = np.asarray(inputs["w_o"], f32)
    kc = np.asarray(inputs["k_cache"], f32)
    vc = np.asarray(inputs["v_cache"], f32)
    ff1 = np.asarray(inputs["w_ff1"], f32)
    ff2 = np.asarray(inputs["w_ff2"], f32)

    def cols(v):
        return np.ascontiguousarray(v.reshape(KT, 128).T)

    x_c, an_c, fn_c = cols(x), cols(an), cols(fn)
    rope_tbl = np.stack(
        [
            np.tile(cos_r, HPC),
            np.tile(sin_r, HPC),
            np.tile(cos_r, HPC) * SCALE,
            np.tile(sin_r, HPC) * SCALE,
        ]
    ).astype(f32)
    eye32 = np.eye(32, dtype=f32)

    in_maps = []
    for c in range(CORES):
        qlo, qhi = c * QC, (c + 1) * QC
        flo, fhi = c * FFC, (c + 1) * FFC
        hlo, hhi = c * HPC, (c + 1) * HPC
        kc_c = kc[:, hlo:hhi, :].reshape(H, QC).astype(NP_BF16)
        vc_c = vc[:, hlo:hhi, :].reshape(H, QC).astype(NP_BF16)
        ff1_c = np.zeros((H, FFCP), dtype=NP_BF16)
        ff1_c[:, :FFC] = ff1[:, flo:fhi].astype(NP_BF16)
        ff2_c = np.zeros((FFKT * 128, H), dtype=NP_BF16)
        ff2_c[:FFC, :] = ff2[flo:fhi, :].astype(NP_BF16)
        in_maps.append(
            {
                "x_cols": x_c,
                "an_cols": an_c,
                "fn_cols": fn_c,
                "rope_tbl": rope_tbl,
                "eye32": eye32,
                "wq": _pack_k8(wq[:, qlo:qhi].astype(NP_BF16)),
                "wk": _pack_k8(wk[:, qlo:qhi].astype(NP_BF16)),
                "wv": _pack_k8(wv[:, qlo:qhi].astype(NP_BF16)),
                "wo": np.ascontiguousarray(
                    wo[qlo:qhi, :].astype(NP_BF16).reshape(4, 128, H)
                ),
                "kc": _pack_k8(kc_c),
                "vc": _pack_k8(vc_c),
                "ff1": np.ascontiguousarray(
                    ff1_c.reshape(8, 4, 128, FFCP).transpose(0, 2, 1, 3)
                ),
                "ff2": np.ascontiguousarray(ff2_c.reshape(FFKT, 128, H)),
            }
        )
    return in_maps


def _assemble(results):
    xnew_cols = results[0]["xnew_out"]  # [128, 32], element (p,t) = vec[t*128+p]
    xnew = np.ascontiguousarray(xnew_cols.T).reshape(-1)
    ff = np.sum(
        np.stack([results[c]["ff_out"] for c in range(CORES)]), axis=0,
        dtype=np.float32,
    )
    return (xnew + ff).astype(np.float32)


def run(inputs, trace=False):
    pos = int(inputs["pos"])
    nc = _build(pos)
    in_maps = _shard(inputs, pos)
    res = run_bass_kernel_spmd(nc, in_maps, core_ids=list(range(CORES)), trace=trace)
    return _assemble(res.results), res


def kernel(**inputs) -> np.ndarray:
    out, _ = run(inputs, trace=False)
    return out
